# revision 1
# baseline (speedup 1.0000x reference)
"""Trainium2 Bass kernel for ArcShapeRadiusConfigVisibleNeighDist.

For each pedestrian i (N=8192):
  heading u_i = normalize(pos_i - past_i)
  over all j: dist_ij = |pos_j - pos_i|, visible iff angle(pos_j-pos_i, u_i)
  in [-35deg, 35deg) and j != i. Output = affine(clip(mean visible dist)).

Key reformulation (no atan2 anywhere):
  visible  <=>  rel . u_i > cos(35deg) * dist  <=>  dot/c > dist
  sq and dot/c are K-small matmuls on the TensorEngine with fp16 hi/lo
  split features (K is free on the PE), giving near-fp32 accuracy at
  full PE speed. G1 (K=10, rows 0-9) and G2 (K=8, rows 32-39) are packed
  into different PE row-groups via tile_position so they run concurrently.

Per 128-query x 2048-j pair of chunks:
  PE:  G1 = sq (+eps) [128,2048], G2 = dot/c (2x [128,1024])
  ACT: dist = sqrt(G1) -> fp16 [128,2048]
  DVE: custom MASKED_SD per 1024: sd = select(G2 > dist, dist, 0), accum -> s
  cnt pass per 2048 (DVE tensor_scalar or ACT Sign, per schedule)
Epilogue: r = clip(s/max(cnt,1) * k + b, 0.5, 4.0); select by indexes.

Sharding: core k owns queries [k*1024, (k+1)*1024), full j set.
"""

import numpy as np

import concourse.bass as bass
import concourse.bacc as bacc
import concourse.mybir as mybir
import concourse.tile as tile
from contextlib import ExitStack
from concourse.alu_op_type import AluOpType
from concourse.bass_utils import run_bass_kernel_spmd
from concourse.dve_uop import DveOpSpec
import concourse.dve_ops as dvo
from concourse.dve_ops import Spec, Src0, Src1, Zero, select, lower, has_src1
from concourse.dve_ops import AluOp as SAluOp

N = 8192
NCORES = 8
Q = N // NCORES            # 1024 queries per core
ITILES = Q // 128          # 8 partition tiles of queries
JCHUNK = 1024
JPAIR = 2 * JCHUNK         # 2048: sqrt/cnt granularity
NJP = N // JPAIR           # 4 j-pairs per i-tile
EPS = 0.005                # sq guard: keeps diag excluded, sqrt input > 0
COS_HALF = float(np.cos(70.0 * np.pi / 180.0 / 2.0))
MIN_R, MAX_R = 0.5, 4.0
MIN_D, MAX_D = 0.2, 5.0
SLOPE = (MAX_R - MIN_R) / (MAX_D - MIN_D)
OFFS = MIN_R - MIN_D * SLOPE

F32 = mybir.dt.float32
FP16 = mybir.dt.float16
ACTF = mybir.ActivationFunctionType
_F16 = np.float16

# count-pass engine per (itile * NJC + jchunk): 't' DVE tensor_scalar+accum,
# 'a' ACT Sign+accum. Balanced so DVE ~= ACT total busy.
NJC = N // JCHUNK
N_T = 27
CNT_SCHED = [('t' if (i * 37) % 64 < N_T else 'a') for i in range(ITILES * NJC)]
JF_SPLIT = 4               # jf DMA'd as column tiles so first matmuls start early


def register_masked_sd():
    """Runtime-register the custom DVE op: out = select(in0 > in1, in1, 0),
    accum_out = sum(out). The per-NEFF uop table is generated from OPS, so
    appending at runtime is sufficient (no firmware change)."""
    name = "MASKED_SD_ANT"
    if name in dvo._SUB_OPCODE_FOR_NAME:
        return getattr(dvo, name)

    def _ref(in0, in1, s0, s1, imm2):
        b = np.where(in0.astype(np.float32) > in1, in1, 0.0).astype(np.float32)
        return b, b.reshape(b.shape[0], -1).sum(axis=-1, keepdims=True)

    spec = Spec(body=select(Src0 > Src1, Src1, Zero), accum=SAluOp.ADD,
                reference=_ref)
    row = max(dvo._SUB_OPCODE_FOR_NAME.values()) + 1
    assert row < 0x20
    dvo._SUB_OPCODE_FOR_NAME[name] = row
    op = dvo.DveOp(name, spec, subdim=False, uops_sha={})
    for ver in ("v3", "v4"):
        s = DveOpSpec(name=name, opcode=row, uops=lower(spec, ver=ver),
                      rd1_en=has_src1(spec))
        op.uops_sha[ver] = s.sha(ver)
    dvo.OPS.append(op)
    dvo.CUSTOM_DVE_SPECS[name] = spec
    setattr(dvo, name, op)
    return op


def _split(x):
    """Split f64 array into fp16 hi + fp16 lo (as f64 of exact fp16 values)."""
    h = x.astype(_F16).astype(np.float64)
    l = (x - h).astype(_F16).astype(np.float64)
    return h, l


def _build_graph():
    masked_sd = register_masked_sd()
    nc = bacc.Bacc("TRN2", target_bir_lowering=False, debug=False,
                   num_devices=NCORES)
    # qf/jf rows 0-9: G1 features; rows 32-39: G2 features (row-group 1)
    qf1_d = nc.dram_tensor("qf1", [10, Q], FP16, kind="ExternalInput")
    qf2_d = nc.dram_tensor("qf2", [8, Q], FP16, kind="ExternalInput")
    jf1_d = nc.dram_tensor("jf1", [10, N], FP16, kind="ExternalInput")
    jf2_d = nc.dram_tensor("jf2", [8, N], FP16, kind="ExternalInput")
    os_d = nc.dram_tensor("out_s", [128, ITILES * NJC], F32,
                          kind="ExternalOutput")
    oct_d = nc.dram_tensor("out_ct", [128, ITILES * NJC], F32,
                           kind="ExternalOutput")
    oca_d = nc.dram_tensor("out_ca", [128, ITILES * NJC], F32,
                           kind="ExternalOutput")

    with tile.TileContext(nc) as tc, ExitStack() as ctx:
        singles = ctx.enter_context(tc.tile_pool(name="singles", bufs=1))
        psum = ctx.enter_context(tc.tile_pool(name="psum", bufs=2, space="PSUM"))
        work = ctx.enter_context(tc.tile_pool(name="work", bufs=6))
        parts = ctx.enter_context(tc.tile_pool(name="parts", bufs=2))
        # (config: chunk_w6 + jf column-split + fp8 msk, A/B-validated)

        qf = singles.tile([40, Q], FP16)
        nc.sync.dma_start(qf[0:10, :], qf1_d[:])
        nc.sync.dma_start(qf[32:40, :], qf2_d[:])
        jw = N // JF_SPLIT
        jfs = []
        for t in range(JF_SPLIT):
            jft = singles.tile([40, jw], FP16, tag=f"jf{t}")
            nc.sync.dma_start(jft[0:10, :], jf1_d[:, t * jw:(t + 1) * jw])
            nc.sync.dma_start(jft[32:40, :], jf2_d[:, t * jw:(t + 1) * jw])
            jfs.append(jft)
        # single-writer accumulator stripes; final math happens on host
        s_all = singles.tile([128, ITILES * NJC], F32)
        ct_all = singles.tile([128, ITILES * NJC], F32)
        ca_all = singles.tile([128, ITILES * NJC], F32)
        nc.vector.memset(ct_all[:], 0.0)
        nc.scalar.activation(ca_all[:], ct_all[:], ACTF.Copy, scale=0.0)

        for it in range(ITILES):
            lhs1 = qf[0:10, bass.ts(it, 128)]
            lhs2 = qf[32:40, bass.ts(it, 128)]
            for jc in range(NJC):
                gi = it * NJC + jc
                g1 = psum.tile([128, JCHUNK], F32, tag="g1")
                g2 = psum.tile([128, JCHUNK], F32, tag="g2")
                for h in range(2):
                    col = jc * JCHUNK + h * 512
                    jft = jfs[col // jw]
                    cl = col % jw
                    hs = slice(h * 512, (h + 1) * 512)
                    nc.tensor.matmul(g1[:, hs], lhs1, jft[0:10, cl:cl + 512],
                                     tile_position=(0, 0))
                    nc.tensor.matmul(g2[:, hs], lhs2, jft[32:40, cl:cl + 512],
                                     tile_position=(32, 0))
                dist = work.tile([128, JCHUNK], FP16, tag="dist")
                nc.scalar.activation(dist[:], g1[:], ACTF.Sqrt)
                sd = work.tile([128, JCHUNK], FP16, tag="sd")
                nc.vector._custom_dve(
                    masked_sd, out=sd[:], in0=g2[:], in1=dist[:],
                    accum_out=s_all[:, gi:gi + 1])
                eng = CNT_SCHED[gi]
                msk = work.tile([128, JCHUNK], mybir.dt.float8e4, tag="msk")
                if eng == 't':
                    nc.vector.tensor_scalar(
                        out=msk[:], in0=sd[:], scalar1=0.0, scalar2=0.0,
                        op0=AluOpType.is_gt, op1=AluOpType.add,
                        accum_out=ct_all[:, gi:gi + 1])
                else:  # 'a'
                    nc.scalar.activation(msk[:], sd[:], ACTF.Sign,
                                         accum_out=ca_all[:, gi:gi + 1])

        nc.sync.dma_start(os_d[:], s_all[:])
        nc.sync.dma_start(oct_d[:], ct_all[:])
        nc.sync.dma_start(oca_d[:], ca_all[:])

    nc.compile()
    return nc


_CACHED_NC = None


def _get_nc():
    global _CACHED_NC
    if _CACHED_NC is None:
        _CACHED_NC = _build_graph()
    return _CACHED_NC


def _prep_inputs(past_ped_positions, ped_positions, indexes, all_radii):
    pos = np.asarray(ped_positions, np.float64)
    past = np.asarray(past_ped_positions, np.float64)
    v = pos - past
    vn = np.hypot(v[:, 0], v[:, 1])
    safe = np.where(vn > 0, vn, 1.0)
    ux = np.where(vn > 0, v[:, 0] / safe, 1.0)
    uy = np.where(vn > 0, v[:, 1] / safe, 0.0)

    px, py = pos[:, 0], pos[:, 1]
    nsq = px * px + py * py
    px_h, px_l = _split(px)
    py_h, py_l = _split(py)
    nsq_h, nsq_l = _split(nsq)
    ones = np.ones(N)
    jf1 = np.stack([px_h, px_l, px_h, py_h, py_l, py_h, ones, ones,
                    nsq_h, nsq_l]).astype(_F16)
    jf2 = jf1[0:8].copy()

    a = ux / COS_HALF
    b = uy / COS_HALF
    w = (ux * px + uy * py) / COS_HALF
    a_h, a_l = _split(a)
    b_h, b_l = _split(b)
    w_h, w_l = _split(w)
    nq_h, nq_l = _split(nsq + EPS)
    qf1_full = np.stack([-2 * px_h, -2 * px_h, -2 * px_l,
                         -2 * py_h, -2 * py_h, -2 * py_l,
                         nq_h, nq_l, ones, ones])  # [10, N]
    qf2_full = np.stack([a_h, a_h, a_l, b_h, b_h, b_l, -w_h, -w_l])  # [8, N]

    # column c of per-core qf holds local query (c % 128) * ITILES + c // 128
    cidx = np.arange(Q)
    perm = (cidx % 128) * ITILES + cidx // 128

    in_maps = []
    for k in range(NCORES):
        sl = slice(k * Q, (k + 1) * Q)
        qf1_core = qf1_full[:, sl][:, perm].astype(_F16)
        qf2_core = qf2_full[:, sl][:, perm].astype(_F16)
        in_maps.append({"qf1": qf1_core, "qf2": qf2_core, "jf1": jf1,
                        "jf2": jf2})
    return in_maps


def _host_epilogue(res_core, idxf_core, radii_core):
    """[128, 64] accumulator stripes -> [1024] final radii for one core.
    idxf_core/radii_core are [128, ITILES] (local query q = p*ITILES + it)."""
    s = np.asarray(res_core["out_s"], np.float64).reshape(
        128, ITILES, NJC).sum(2)
    c = (np.asarray(res_core["out_ct"], np.float64)
         + np.asarray(res_core["out_ca"], np.float64)).reshape(
             128, ITILES, NJC).sum(2)
    mean = (s / np.maximum(c, 1.0)).astype(np.float32)
    r = np.clip(mean * np.float32(SLOPE) + np.float32(OFFS), MIN_R, MAX_R)
    fin = radii_core + idxf_core * (r - radii_core)
    return fin.astype(np.float32).reshape(Q)


def kernel(past_ped_positions, ped_positions, indexes, all_radii,
           _trace=False, _trace_kwargs=None):
    nc = _get_nc()
    in_maps = _prep_inputs(past_ped_positions, ped_positions, indexes,
                           all_radii)
    kw = {}
    if _trace:
        kw = {"trace": True}
        if _trace_kwargs:
            kw.update(_trace_kwargs)
    res = run_bass_kernel_spmd(nc, in_maps, list(range(NCORES)), **kw)
    idxf = np.asarray(indexes).astype(np.float32)
    radii = np.asarray(all_radii, np.float32)
    outs = []
    for k in range(NCORES):
        sl = slice(k * Q, (k + 1) * Q)
        outs.append(_host_epilogue(res.results[k],
                                   idxf[sl].reshape(128, ITILES),
                                   radii[sl].reshape(128, ITILES)))
    out = np.concatenate(outs)
    if _trace:
        kernel.last_results = res
    return out



# revision 2
# speedup vs baseline: 1.6222x; 1.6222x over previous
"""Trainium2 Bass kernel for ArcShapeRadiusConfigVisibleNeighDist.

For each pedestrian i (N=8192):
  heading u_i = normalize(pos_i - past_i)
  over all j: dist_ij = |pos_j - pos_i|, visible iff angle(pos_j-pos_i, u_i)
  in [-35deg, 35deg) and j != i. Output = affine(clip(mean visible dist)).

Key reformulation (no atan2 anywhere):
  visible  <=>  rel . u_i > cos(35deg) * dist  <=>  dot/c > dist
  sq and dot/c are K-small matmuls on the TensorEngine with fp16 hi/lo
  split features (K is free on the PE), giving near-fp32 accuracy at
  full PE speed. G1 (K=10, rows 0-9) and G2 (K=8, rows 32-39) are packed
  into different PE row-groups via tile_position so they run concurrently.

Per 128-query x 1024-j chunk (single fused vector pass per element):
  PE:  G1 = sq (+eps) [128,1024], G2 = dot/c [128,1024]
  ACT: dist = sqrt(G1) -> fp16 [128,1024]
  DVE: custom MASKED_SDC: b = select(G2 > dist, dist + ENC_C, 0),
       accum -> A = ENC_C*cnt + s  (one accumulator carries BOTH the
       visible count and the visible-distance sum; per-chunk s < 2^17
       so the host separates them with a floor-divide).
Host epilogue: cnt = floor((A+64)/ENC_C); s = A - ENC_C*cnt per chunk,
  summed over chunks; r = clip(s/max(cnt,1) * k + b, 0.5, 4.0);
  select by indexes.

Sharding: core k owns queries [k*1024, (k+1)*1024), full j set.
"""

import numpy as np

import concourse.bass as bass
import concourse.bacc as bacc
import concourse.mybir as mybir
import concourse.tile as tile
from contextlib import ExitStack
from concourse.bass_utils import run_bass_kernel_spmd
from concourse.dve_uop import DveOpSpec
import concourse.dve_ops as dvo
from concourse.dve_ops import Spec, Src0, Src1, Zero, C1, select, lower, has_src1
from concourse.dve_ops import AluOp as SAluOp

N = 8192
NCORES = 8
Q = N // NCORES            # 1024 queries per core
ITILES = Q // 128          # 8 partition tiles of queries
JCHUNK = 1024
NJC = N // JCHUNK          # 8 j-chunks per i-tile
EPS = 0.005                # sq guard: keeps diag excluded, sqrt input > 0
COS_HALF = float(np.cos(70.0 * np.pi / 180.0 / 2.0))
MIN_R, MAX_R = 0.5, 4.0
MIN_D, MAX_D = 0.2, 5.0
SLOPE = (MAX_R - MIN_R) / (MAX_D - MIN_D)
OFFS = MIN_R - MIN_D * SLOPE
ENC_C = 131072.0           # 2^17: per-chunk s < 1024*dmax ~ 98e3 < 2^17

F32 = mybir.dt.float32
FP16 = mybir.dt.float16
ACTF = mybir.ActivationFunctionType
_F16 = np.float16

JF_SPLIT = 4               # jf DMA'd as column tiles so first matmuls start early


def register_masked_sdc():
    """Runtime-register the fused DVE op:
    out = select(in0 > in1, in1 + s1, 0), accum_out = sum(out).
    With s1 = ENC_C the accumulator encodes ENC_C*count + sum(dist) in one
    fp32 lane. The per-NEFF uop table is generated from OPS, so appending
    at runtime is sufficient (no firmware change)."""
    name = "MASKED_SDC_ANT"
    if name in dvo._SUB_OPCODE_FOR_NAME:
        return getattr(dvo, name)

    def _ref(in0, in1, s0, s1, imm2):
        b = np.where(in0.astype(np.float32) > in1,
                     in1.astype(np.float32) + np.float32(s1),
                     0.0).astype(np.float32)
        return b, b.reshape(b.shape[0], -1).sum(axis=-1, keepdims=True)

    spec = Spec(body=select(Src0 > Src1, Src1 + C1, Zero), accum=SAluOp.ADD,
                reference=_ref)
    row = max(dvo._SUB_OPCODE_FOR_NAME.values()) + 1
    assert row < 0x20
    dvo._SUB_OPCODE_FOR_NAME[name] = row
    op = dvo.DveOp(name, spec, subdim=False, uops_sha={})
    for ver in ("v3", "v4"):
        s = DveOpSpec(name=name, opcode=row, uops=lower(spec, ver=ver),
                      rd1_en=has_src1(spec))
        op.uops_sha[ver] = s.sha(ver)
    dvo.OPS.append(op)
    dvo.CUSTOM_DVE_SPECS[name] = spec
    setattr(dvo, name, op)
    return op


def _split(x):
    """Split f64 array into fp16 hi + fp16 lo (as f64 of exact fp16 values)."""
    h = x.astype(_F16).astype(np.float64)
    l = (x - h).astype(_F16).astype(np.float64)
    return h, l


def _build_graph():
    masked_sdc = register_masked_sdc()
    nc = bacc.Bacc("TRN2", target_bir_lowering=False, debug=False,
                   num_devices=NCORES)
    # qf/jf rows 0-9: G1 features; rows 32-39: G2 features (row-group 1)
    qf1_d = nc.dram_tensor("qf1", [10, Q], FP16, kind="ExternalInput")
    qf2_d = nc.dram_tensor("qf2", [8, Q], FP16, kind="ExternalInput")
    jf1_d = nc.dram_tensor("jf1", [10, N], FP16, kind="ExternalInput")
    jf2_d = nc.dram_tensor("jf2", [8, N], FP16, kind="ExternalInput")
    oa_d = nc.dram_tensor("out_a", [128, ITILES * NJC], F32,
                          kind="ExternalOutput")

    with tile.TileContext(nc) as tc, ExitStack() as ctx:
        singles = ctx.enter_context(tc.tile_pool(name="singles", bufs=1))
        psum = ctx.enter_context(tc.tile_pool(name="psum", bufs=2, space="PSUM"))
        work = ctx.enter_context(tc.tile_pool(name="work", bufs=4))

        qf = singles.tile([40, Q], FP16)
        nc.sync.dma_start(qf[0:10, :], qf1_d[:])
        nc.sync.dma_start(qf[32:40, :], qf2_d[:])
        jw = N // JF_SPLIT
        jfs = []
        for t in range(JF_SPLIT):
            jft = singles.tile([40, jw], FP16, tag=f"jf{t}")
            nc.sync.dma_start(jft[0:10, :], jf1_d[:, t * jw:(t + 1) * jw])
            nc.sync.dma_start(jft[32:40, :], jf2_d[:, t * jw:(t + 1) * jw])
            jfs.append(jft)
        # single-writer accumulator stripes; final math happens on host
        a_all = singles.tile([128, ITILES * NJC], F32)

        for it in range(ITILES):
            lhs1 = qf[0:10, bass.ts(it, 128)]
            lhs2 = qf[32:40, bass.ts(it, 128)]
            for jc in range(NJC):
                gi = it * NJC + jc
                g1 = psum.tile([128, JCHUNK], F32, tag="g1")
                g2 = psum.tile([128, JCHUNK], F32, tag="g2")
                for h in range(2):
                    col = jc * JCHUNK + h * 512
                    jft = jfs[col // jw]
                    cl = col % jw
                    hs = slice(h * 512, (h + 1) * 512)
                    nc.tensor.matmul(g1[:, hs], lhs1, jft[0:10, cl:cl + 512],
                                     tile_position=(0, 0))
                    nc.tensor.matmul(g2[:, hs], lhs2, jft[32:40, cl:cl + 512],
                                     tile_position=(32, 0))
                dist = work.tile([128, JCHUNK], FP16, tag="dist")
                nc.scalar.activation(dist[:], g1[:], ACTF.Sqrt)
                junk = work.tile([128, JCHUNK], mybir.dt.float8e4, tag="jk")
                nc.vector._custom_dve(
                    masked_sdc, out=junk[:], in0=g2[:], in1=dist[:],
                    s1=ENC_C, accum_out=a_all[:, gi:gi + 1])

        nc.sync.dma_start(oa_d[:], a_all[:])

    nc.compile()
    return nc


_CACHED_NC = None


def _get_nc():
    global _CACHED_NC
    if _CACHED_NC is None:
        _CACHED_NC = _build_graph()
    return _CACHED_NC


def _prep_inputs(past_ped_positions, ped_positions, indexes, all_radii):
    pos = np.asarray(ped_positions, np.float64)
    past = np.asarray(past_ped_positions, np.float64)
    v = pos - past
    vn = np.hypot(v[:, 0], v[:, 1])
    safe = np.where(vn > 0, vn, 1.0)
    ux = np.where(vn > 0, v[:, 0] / safe, 1.0)
    uy = np.where(vn > 0, v[:, 1] / safe, 0.0)

    px, py = pos[:, 0], pos[:, 1]
    nsq = px * px + py * py
    px_h, px_l = _split(px)
    py_h, py_l = _split(py)
    nsq_h, nsq_l = _split(nsq)
    ones = np.ones(N)
    jf1 = np.stack([px_h, px_l, px_h, py_h, py_l, py_h, ones, ones,
                    nsq_h, nsq_l]).astype(_F16)
    jf2 = jf1[0:8].copy()

    a = ux / COS_HALF
    b = uy / COS_HALF
    w = (ux * px + uy * py) / COS_HALF
    a_h, a_l = _split(a)
    b_h, b_l = _split(b)
    w_h, w_l = _split(w)
    nq_h, nq_l = _split(nsq + EPS)
    qf1_full = np.stack([-2 * px_h, -2 * px_h, -2 * px_l,
                         -2 * py_h, -2 * py_h, -2 * py_l,
                         nq_h, nq_l, ones, ones])  # [10, N]
    qf2_full = np.stack([a_h, a_h, a_l, b_h, b_h, b_l, -w_h, -w_l])  # [8, N]

    # column c of per-core qf holds local query (c % 128) * ITILES + c // 128
    cidx = np.arange(Q)
    perm = (cidx % 128) * ITILES + cidx // 128

    in_maps = []
    for k in range(NCORES):
        sl = slice(k * Q, (k + 1) * Q)
        qf1_core = qf1_full[:, sl][:, perm].astype(_F16)
        qf2_core = qf2_full[:, sl][:, perm].astype(_F16)
        in_maps.append({"qf1": qf1_core, "qf2": qf2_core, "jf1": jf1,
                        "jf2": jf2})
    return in_maps


def _host_epilogue(res_core, idxf_core, radii_core):
    """[128, 64] encoded accumulator stripes -> [1024] final radii for one
    core. idxf_core/radii_core are [128, ITILES] (local query
    q = p*ITILES + it). Each accumulator lane holds ENC_C*cnt + s for one
    (query, j-chunk); s < ENC_C so floor-divide separates them (+64 absorbs
    downward fp accumulation error in near-empty chunks)."""
    A = np.asarray(res_core["out_a"], np.float64).reshape(128, ITILES, NJC)
    cnt_c = np.floor((A + 64.0) / ENC_C)
    s_c = A - ENC_C * cnt_c
    c = cnt_c.sum(2)
    s = s_c.sum(2)
    mean = (s / np.maximum(c, 1.0)).astype(np.float32)
    r = np.clip(mean * np.float32(SLOPE) + np.float32(OFFS), MIN_R, MAX_R)
    fin = radii_core + idxf_core * (r - radii_core)
    return fin.astype(np.float32).reshape(Q)


def kernel(past_ped_positions, ped_positions, indexes, all_radii,
           _trace=False, _trace_kwargs=None):
    nc = _get_nc()
    in_maps = _prep_inputs(past_ped_positions, ped_positions, indexes,
                           all_radii)
    kw = {}
    if _trace:
        kw = {"trace": True}
        if _trace_kwargs:
            kw.update(_trace_kwargs)
    res = run_bass_kernel_spmd(nc, in_maps, list(range(NCORES)), **kw)
    idxf = np.asarray(indexes).astype(np.float32)
    radii = np.asarray(all_radii, np.float32)
    outs = []
    for k in range(NCORES):
        sl = slice(k * Q, (k + 1) * Q)
        outs.append(_host_epilogue(res.results[k],
                                   idxf[sl].reshape(128, ITILES),
                                   radii[sl].reshape(128, ITILES)))
    out = np.concatenate(outs)
    if _trace:
        kernel.last_results = res
    return out
